# revision 35
# baseline (speedup 1.0000x reference)
"""Trainium2 Bass kernel: KV-memory retrieval (pool -> cosine kNN -> softmax gather).

Strategy (8 cores): shard the 65536-slot memory across cores (8192 keys/values
each) and the 256-image batch across cores (32 each) for pooling + output.
Keys are pre-transposed host-side to [C, M] so each core DMAs c-partitioned
kT tiles directly (no PE transposes for keys).

Per core, single SPMD launch:
  1. pool its x shard -> qT columns; two chunked AllGathers (first 16
     batches AG'd while the second half of x still streams in).
     Batch-tile permutation: tile A col r*16+j == global batch r*32+j,
     tile B col r*16+j == global batch r*32+16+j (undone at the mrow
     scatter before ReduceScatter).
  2. stream kT blocks [128c, 512m]: squares (ACT/DVE), norm via f32r
     ones-matmul over partitions, sqrt/recip, PE row-broadcast, DVE
     prescale -> normalized kTn (f32r); f32r matmul1 qT.T @ kTn ->
     sim [256, 8192]; per-block top-8 candidates (max8)
  3. local top-32 -> AllGather candidates -> global top-32 (sorted),
     threshold t, softmax stats gmax / Z (exp with per-partition
     scale/bias)
  4. dense w = exp(sim*rinv + bias) * (sim >= t)  (1/Z folded into bias),
     stored f32r
  5. matmul2 (f32r): values.T @ w -> partial matched.T; values are
     bitcast f32->f32r (no copy)
  6. transpose -> [256, 512], ReduceScatter(add) -> own batch shard
  7. broadcast over 784 spatial positions, DMA out [32, 512, 784]

f32r (single-pass fp32 on the PE) is safe here: measured sim noise ~1e-6
in cos units vs ~4e-4 gaps between rank 32/33. Selection is done on raw
r = q_sum . k_norm (scale-invariant per batch row); 1/||q|| enters only
through the exp scale. Mean /784 cancels everywhere.
"""

import math

import numpy as np

import concourse.bacc as bacc
import concourse.mybir as mybir
import concourse.tile as tile
from concourse.bass import ts
from concourse.bass_utils import run_bass_kernel_spmd
from concourse.masks import make_identity

F32 = mybir.dt.float32
F32R = mybir.dt.float32r
AF = mybir.ActivationFunctionType
ALU = mybir.AluOpType

N_CORES = 8
NEG = -3.0e38


def build(B=256, C=512, HW=784, M=65536, K=32, n_cores=N_CORES, mb=512):
    """Build + bacc-compile the SPMD program. Returns nc."""
    BS = B // n_cores          # batches per core
    HB = BS // 2               # half-batch chunk for split AllGather
    MS = M // n_cores          # memory slots per core
    CT = C // 128              # channel tiles (also contraction tiles)
    BT = B // 128 if B >= 128 else 1
    BTW = 128 if B >= 128 else B   # batch-tile width
    assert B % BTW == 0 and C % 128 == 0 and M % (n_cores * mb) == 0
    NMB = MS // mb             # key blocks per core
    R = math.ceil(K / 8)       # max8 rounds for exact top-K
    KPB = 8                    # top-8 per 512-block (validated sufficient)
    MT = MS // 128             # value tiles
    RG = [list(range(n_cores))]
    CC_AS = "Shared" if n_cores > 4 else "Local"

    nc = bacc.Bacc("TRN2", target_bir_lowering=False, debug=False,
                   num_devices=n_cores)

    xs = nc.dram_tensor("xs", [BS, C, HW], F32, kind="ExternalInput").ap()
    keysT = nc.dram_tensor("keysT", [C, MS], F32, kind="ExternalInput").ap()
    vals = nc.dram_tensor("vals", [MS, C], F32, kind="ExternalInput").ap()
    out = nc.dram_tensor("out", [BS, C, HW], F32, kind="ExternalOutput").ap()

    with tile.TileContext(nc) as tc:
        with (
            tc.tile_pool(name="consts", bufs=1) as consts,
            tc.tile_pool(name="persist", bufs=1) as persist,
            tc.tile_pool(name="dram", bufs=1, space="DRAM") as dram,
        ):
            identity = consts.tile([128, 128], F32)
            make_identity(nc, identity)
            identity_r = consts.tile([128, 128], F32R)
            nc.vector.tensor_copy(identity_r, identity)
            ones_col = consts.tile([128, 1], F32)
            nc.vector.memset(ones_col, 1.0)
            ones_col_r = consts.tile([128, 1], F32R)
            nc.vector.tensor_copy(ones_col_r, ones_col)
            ones_row = consts.tile([1, 128], F32)
            nc.vector.memset(ones_row, 1.0)
            ones_row_r = consts.tile([1, 128], F32R)
            nc.vector.tensor_copy(ones_row_r, ones_row)
            ones_hw = consts.tile([128, HW], F32)
            nc.vector.memset(ones_hw, 1.0)
            identity_h = consts.tile([128, 128], mybir.dt.bfloat16)
            nc.vector.tensor_copy(identity_h, identity)

            sim = [persist.tile([BTW, MS], F32, name=f"sim{i}")
                   for i in range(BT)]
            qTt = persist.tile([128, CT, B], F32, name="qTt")
            qT = [qTt[:, i] for i in range(CT)]
            qTr = persist.tile([128, CT, B], F32R, name="qTr")
            qTl = [persist.tile([128, BS], F32, name=f"qTl{i}")
                   for i in range(CT)]
            cand = [persist.tile([BTW, NMB * KPB], F32, name=f"cand{i}")
                    for i in range(BT)]
            rinv = [persist.tile([BTW, 1], F32, name=f"rinv{i}")
                    for i in range(BT)]
            bias2 = [persist.tile([BTW, 1], F32, name=f"bias2{i}")
                     for i in range(BT)]
            g32 = [persist.tile([BTW, R * 8], F32, name=f"g32{i}")
                   for i in range(BT)]
            mrow = [persist.tile([BTW, C], F32, name=f"mrow{i}")
                    for i in range(BT)]
            mTmy = [persist.tile([128, BS], F32, name=f"mTmy{i}")
                    for i in range(CT)]

            # AllGather staging (queries, two half-batch chunks)
            qag_in = [dram.tile([C, HB], F32, name=f"qag_in{h}")
                      for h in range(2)]
            qag_out = [dram.tile([n_cores, C, HB], F32, addr_space=CC_AS,
                                 name=f"qag_out{h}") for h in range(2)]

            def ag_queries(h):
                # stage local half-chunk h and AllGather it (gpsimd only; the
                # qTt unpack is emitted separately so it never blocks the
                # ACT queue mid-pooling)
                for ct in range(CT):
                    nc.scalar.dma_start(
                        out=qag_in[h][ts(ct, 128), :],
                        in_=qTl[ct][:, h * HB:(h + 1) * HB])
                nc.gpsimd.collective_compute(
                    "AllGather", ALU.bypass, replica_groups=RG,
                    ins=[qag_in[h].opt()], outs=[qag_out[h].opt()])

            def unpack_queries(h):
                # qTt tile-h columns [h*128 + r*HB : ...] (permuted layout)
                for r in range(n_cores):
                    nc.scalar.dma_start(
                        out=qTt[:, :, h * BTW + r * HB:
                                h * BTW + (r + 1) * HB],
                        in_=qag_out[h][r].rearrange("(ct p) b -> p ct b",
                                                    p=128))
                nc.vector.tensor_copy(
                    qTr[:, :, h * BTW:(h + 1) * BTW],
                    qTt[:, :, h * BTW:(h + 1) * BTW])

            # ---------------- Phase P: pool x -> qT local ----------------
            with (
                tc.tile_pool(name="poolP", bufs=3) as pP,
            ):
                hw_a = 0
                for a in range(int(math.isqrt(HW)), 1, -1):
                    if HW % a == 0:
                        hw_a = a
                        break
                CTH = CT // 2
                for b in range(BS):
                    if b % 2 == 0:
                        xt2 = pP.tile([128, 2, CT, HW], F32, tag="xt2",
                                      bufs=3)
                        nc.sync.dma_start(
                            out=xt2,
                            in_=xs[b:b + 2].rearrange(
                                "b (ct p) hw -> p b ct hw", p=128))
                    xt = xt2[:, b % 2]
                    if hw_a > 1:
                        xp = pP.tile([128, CTH, HW // hw_a], F32, tag="xp")
                        nc.vector.tensor_reduce(
                            out=xp,
                            in_=xt[:, 0:CTH].rearrange(
                                "p ct (a b) -> p ct a b", a=HW // hw_a),
                            axis=mybir.AxisListType.X, op=ALU.add)
                        xq = pP.tile([128, CTH], F32, tag="xq")
                        nc.vector.tensor_reduce(
                            out=xq, in_=xp,
                            axis=mybir.AxisListType.X, op=ALU.add)
                    else:
                        xq = pP.tile([128, CTH], F32, tag="xq")
                        nc.vector.tensor_reduce(
                            out=xq, in_=xt[:, 0:CTH],
                            axis=mybir.AxisListType.X, op=ALU.add)
                    for ct in range(CTH):
                        nc.vector.tensor_copy(qTl[ct][:, b:b + 1],
                                              xq[:, ct:ct + 1])
                    for ct in range(CTH, CT):
                        xsc = pP.tile([128, HW], F32, tag="xsc")
                        nc.scalar.activation(
                            xsc, xt[:, ct], AF.Copy,
                            accum_out=qTl[ct][:, b:b + 1])
                    if b == HB - 1:
                        ag_queries(0)   # first-half AG rides under x DMA
                ag_queries(1)

            # ---------------- Phase K: keys -> sim + block candidates -----
            # value prefetch pool spans K+W so DMA never starves after keys
            VB = 4                      # value tiles per DMA
            CW = 4                      # wexp chunk width (value tiles)
            pV_cm = tc.tile_pool(name="poolV", bufs=2)
            pV = pV_cm.__enter__()
            vgroups = {}

            def issue_vals(g):
                vtb = pV.tile([128, VB, C], F32R, tag="vtb", bufs=5)
                nc.sync.dma_start(
                    out=vtb,
                    in_=vals[g * VB * 128:(g + 1) * VB * 128].rearrange(
                        "(v p) c -> p v c", p=128).bitcast(F32R))
                vgroups[g] = vtb

            with (
                tc.tile_pool(name="poolK", bufs=2) as pK,
                tc.tile_pool(name="psumK", bufs=1, space="PSUM") as psK,
            ):
                ktbs = {}
                psbSs = {}

                def key_prep(mbi):
                    # load kT block [128, CT, mb] (c-partitioned, direct),
                    # typed f32r via bitcast (PE truncates internally; the
                    # per-element noise averages out in the dot product)
                    ktb = pK.tile([128, CT, mb], F32R, tag="ktb", bufs=6)
                    nc.sync.dma_start(
                        out=ktb,
                        in_=keysT[:, mbi * mb:(mbi + 1) * mb].rearrange(
                            "(ct p) m -> p ct m", p=128).bitcast(F32R))
                    ktbs[mbi] = ktb
                    # squares -> ksq (f32r-rounded, so the f32r norm matmul
                    # sums exactly-representable terms: ~8e-6 norm error)
                    ksq = pK.tile([128, CT, mb], F32R, tag="ksq", bufs=2)
                    for ct in range(CT):
                        if ct == 0:
                            nc.vector.tensor_mul(ksq[:, ct], ktb[:, ct],
                                                 ktb[:, ct])
                        else:
                            nc.scalar.square(ksq[:, ct], ktb[:, ct])
                    # norms^2: contract partitions via ones f32r matmul
                    psn = psK.tile([1, mb], F32, tag="psn", bufs=2)
                    for ct in range(CT):
                        nc.tensor.matmul(psn, lhsT=ones_col_r,
                                         rhs=ksq[:, ct],
                                         start=(ct == 0), stop=(ct == CT - 1),
                                         skip_group_check=True)
                    nrow = pK.tile([1, mb], F32, tag="nrow", bufs=2)
                    nc.scalar.sqrt(nrow, psn)
                    rrow = pK.tile([1, mb], F32, tag="rrow", bufs=2)
                    nc.vector.reciprocal(rrow, nrow)
                    # broadcast rinv row to 128 partitions on the PE in FULL
                    # fp32 (1.0 * x is exact; f32r here would round the
                    # per-key scale and flip rank-32/33 boundaries)
                    psb = psK.tile([128, mb], F32, tag="psb", bufs=2)
                    nc.tensor.matmul(psb, lhsT=ones_row, rhs=rrow,
                                     start=True, stop=True,
                                     skip_group_check=True)
                    psbS = pK.tile([128, mb], F32, tag="psbS", bufs=6)
                    nc.scalar.copy(psbS, psb)
                    psbSs[mbi] = psbS

                def sim_block(mbi, bts):
                    ktb = ktbs[mbi]
                    psbS = psbSs[mbi]
                    for bt in bts:
                        psim = psK.tile([BTW, mb], F32, tag="psim", bufs=4)
                        for dt in range(CT):
                            nc.tensor.matmul(
                                psim, lhsT=qTr[:, dt, ts(bt, BTW)],
                                rhs=ktb[:, dt],
                                start=(dt == 0), stop=(dt == CT - 1),
                                skip_group_check=True)
                        # evict + normalize by 1/||k|| in one DVE op
                        sblk = sim[bt][:, ts(mbi, mb)]
                        nc.vector.tensor_mul(sblk, psim, psbS)
                        nc.vector.max(
                            cand[bt][:, mbi * KPB:mbi * KPB + KPB], sblk)

                # tile A (first half-chunk batches) is unblocked by AG1a
                # well before AG1b lands; lead with A-only blocks so the PE
                # queue doesn't stall on AG1b, and unpack chunk 1 only after
                # the lead blocks' ACT work so the AG1b wait doesn't block
                # key_prep. ktb/psbS bufs cover the LEAD+prefetch window to
                # keep slot-recycle acyclic.
                LEAD = 3
                PF = 2
                unpack_queries(0)
                for mbi in range(LEAD + PF):
                    key_prep(mbi)
                for mbi in range(LEAD):
                    sim_block(mbi, [0])
                unpack_queries(1)
                for mbi in range(LEAD):
                    sim_block(mbi, [1] if BT > 1 else [])
                for mbi in range(LEAD, NMB):
                    if mbi + PF < NMB:
                        key_prep(mbi + PF)
                    if mbi >= NMB - 4:
                        issue_vals(mbi - (NMB - 4))
                    sim_block(mbi, list(range(BT)))
                ktbs.clear()
                psbSs.clear()

            # ---------------- Phase Q: query norms ----------------
            with (
                tc.tile_pool(name="poolQ", bufs=2) as pQ,
                tc.tile_pool(name="psumQ", bufs=1, space="PSUM") as psQ,
            ):
                psum_ssq = psQ.tile([1, B], F32, tag="ssq")
                for ct in range(CT):
                    qsq = pQ.tile([128, B], F32R, tag="qsq")
                    nc.scalar.square(qsq, qT[ct])
                    nc.tensor.matmul(psum_ssq, lhsT=ones_col_r, rhs=qsq,
                                     start=(ct == 0), stop=(ct == CT - 1))
                qn_row = pQ.tile([1, B], F32, tag="qn_row", bufs=1)
                nc.scalar.sqrt(qn_row, psum_ssq)
                ri_row = pQ.tile([1, B], F32, tag="ri_row", bufs=1)
                nc.vector.reciprocal(ri_row, qn_row)
                for bt in range(BT):
                    psum_rt = psQ.tile([BTW, 1], F32, tag="rt")
                    nc.tensor.matmul(
                        psum_rt, lhsT=ri_row[0:1, ts(bt, BTW)],
                        rhs=ones_col[0:1, 0:1], start=True, stop=True)
                    nc.vector.tensor_copy(rinv[bt], psum_rt)

            # ---------------- Phase G + W: top-K stats, dense matmul2 -----
            cd_in = dram.tile([B, K], F32)
            cd_out = dram.tile([n_cores, B, K], F32, addr_space=CC_AS)
            with (
                tc.tile_pool(name="poolW", bufs=2) as pW,
                tc.tile_pool(name="psumW", bufs=1, space="PSUM") as psW,
                tc.tile_pool(name="poolG", bufs=1) as pG,
            ):
                # G1: local top-K per bt -> AllGather (bt chains interleaved
                # so the DVE pipeline stays full)
                locs = [pG.tile([BTW, R * 8], F32, tag=f"loc{bt}",
                                name=f"loc{bt}") for bt in range(BT)]
                scr2s = [pG.tile([BTW, NMB * KPB], F32, tag=f"scr2{bt}",
                                 name=f"scr2{bt}") for bt in range(BT)]
                curs = [cand[bt] for bt in range(BT)]
                for r in range(R):
                    for bt in range(BT):
                        nc.vector.max(locs[bt][:, r * 8:(r + 1) * 8],
                                      curs[bt])
                    if r < R - 1:
                        for bt in range(BT):
                            nc.vector.match_replace(
                                scr2s[bt],
                                in_to_replace=locs[bt][:, r * 8:(r + 1) * 8],
                                in_values=curs[bt], imm_value=NEG)
                            curs[bt] = scr2s[bt]
                for bt in range(BT):
                    nc.sync.dma_start(out=cd_in[ts(bt, BTW), :],
                                      in_=locs[bt][:, 0:K])
                nc.gpsimd.collective_compute(
                    "AllGather", ALU.bypass, replica_groups=RG,
                    ins=[cd_in.opt()], outs=[cd_out.opt()])

                issue_vals(4)
                issue_vals(5)

                # G2: global top-K + softmax stats. bt chains interleaved;
                # Exp/Ln activations grouped to avoid ACT table thrash.
                gcs, scr3s = [], []
                for bt in range(BT):
                    gc = pG.tile([BTW, n_cores * K], F32, tag=f"gc{bt}",
                                 name=f"gc{bt}")
                    nc.scalar.dma_start(
                        out=gc,
                        in_=cd_out[:, ts(bt, BTW), :].rearrange(
                            "r b k -> b r k"))
                    gcs.append(gc)
                    scr3s.append(pG.tile([BTW, n_cores * K], F32,
                                         tag=f"scr3{bt}", name=f"scr3{bt}"))
                curs = gcs
                for r in range(R):
                    for bt in range(BT):
                        nc.vector.max(g32[bt][:, r * 8:(r + 1) * 8],
                                      curs[bt])
                    if r < R - 1:
                        for bt in range(BT):
                            nc.vector.match_replace(
                                scr3s[bt],
                                in_to_replace=g32[bt][:, r * 8:(r + 1) * 8],
                                in_values=curs[bt], imm_value=NEG)
                        curs = scr3s
                # stats: nb = -gmax*rinv ; Z = sum exp((g - gmax)*rinv)
                nbs, zzs = [], []
                for bt in range(BT):
                    nb = pG.tile([BTW, 1], F32, tag=f"nb{bt}",
                                 name=f"nb{bt}")
                    nc.vector.tensor_mul(nb, g32[bt][:, 0:1], rinv[bt])
                    nc.vector.tensor_scalar_mul(nb, nb, -1.0)
                    nbs.append(nb)
                for bt in range(BT):
                    ex = pG.tile([BTW, K], F32, tag="ex")
                    zz = pG.tile([BTW, 1], F32, tag=f"zz{bt}",
                                 name=f"zz{bt}")
                    nc.scalar.activation(ex, g32[bt][:, 0:K], AF.Exp,
                                         bias=nbs[bt], scale=rinv[bt],
                                         accum_out=zz)
                    zzs.append(zz)
                lnzs = []
                for bt in range(BT):
                    lnz = pG.tile([BTW, 1], F32, tag=f"lnz{bt}",
                                  name=f"lnz{bt}")
                    nc.scalar.activation(lnz, zzs[bt], AF.Ln)
                    lnzs.append(lnz)
                for bt in range(BT):
                    nc.vector.tensor_sub(bias2[bt], nbs[bt], lnzs[bt])

                # W: dense weights + matmul2
                pmB = [psW.tile([BTW, C], F32, tag=f"pmB{bt}",
                                name=f"pmB{bt}") for bt in range(BT)]
                for mt in range(MT):
                    g = mt // VB
                    if mt % VB == 0 and g + 3 not in vgroups and \
                            (g + 3) * VB * 128 < MS:
                        issue_vals(g + 3)
                    if mt % CW == 0:
                        # weights quantized to bf16 after the f32 threshold
                        # compare: 2^-9 relative on softmax weights is far
                        # below the tolerance, and bf16 transposes run the
                        # PE at 1 cyc/row with fast (FWL) weight loads.
                        weF = [pW.tile([BTW, CW * 128], F32R, tag=f"weF{bt}",
                                       bufs=2, name=f"weF{bt}")
                               for bt in range(BT)]
                        we = [pW.tile([BTW, CW * 128], mybir.dt.bfloat16,
                                      tag=f"we{bt}", bufs=2,
                                      name=f"we{bt}")
                              for bt in range(BT)]
                        for bt in range(BT):
                            schunk = sim[bt][:, mt * 128:(mt + CW) * 128]
                            nc.scalar.activation(weF[bt], schunk, AF.Exp,
                                                 bias=bias2[bt],
                                                 scale=rinv[bt])
                            nc.vector.scalar_tensor_tensor(
                                out=we[bt], in0=schunk,
                                scalar=g32[bt][:, K - 1:K], in1=weF[bt],
                                op0=ALU.is_ge, op1=ALU.mult)
                    vt = vgroups[g][:, mt % VB]
                    off = (mt % CW) * 128
                    pwt = psW.tile([128, B], mybir.dt.bfloat16, tag="pwt",
                                   bufs=4)
                    for bt in range(BT):
                        nc.tensor.matmul(
                            pwt[:, ts(bt, BTW)],
                            lhsT=we[bt][:, off:off + 128],
                            rhs=identity_h[0:BTW, 0:BTW], is_transpose=True,
                            start=True, stop=True, skip_group_check=True)
                    wT = pW.tile([128, B], F32R, tag="wT", bufs=3)
                    if mt % 2 == 0:
                        nc.vector.tensor_copy(wT, pwt)
                    else:
                        nc.scalar.copy(wT, pwt)
                    for bt in range(BT):
                        nc.tensor.matmul(
                            pmB[bt], lhsT=wT[:, ts(bt, BTW)], rhs=vt,
                            start=(mt == 0), stop=(mt == MT - 1),
                            skip_group_check=True)
                for bt in range(BT):
                    nc.any.tensor_copy(mrow[bt], pmB[bt])
            pV_cm.__exit__(None, None, None)

            # ---------------- Phase O: reduce-scatter + broadcast out -----
            mb_dram = dram.tile([B, C], F32)
            rs_out = dram.tile([BS, C], F32)
            with (
                tc.tile_pool(name="poolO", bufs=2) as pO,
                tc.tile_pool(name="psumO", bufs=1, space="PSUM") as psO,
            ):
                # un-permute: tile bt row r*HB+j  ->  global row r*BS+bt*HB+j
                for bt in range(BT):
                    for r in range(n_cores):
                        nc.scalar.dma_start(
                            out=mb_dram[r * BS + bt * HB:
                                        r * BS + (bt + 1) * HB, :],
                            in_=mrow[bt][r * HB:(r + 1) * HB, :])
                nc.gpsimd.collective_compute(
                    "ReduceScatter", ALU.add, replica_groups=RG,
                    ins=[mb_dram.opt()], outs=[rs_out.opt()])
                mmy = pO.tile([BS, C], F32, tag="mmy", bufs=1)
                nc.scalar.dma_start(out=mmy, in_=rs_out)
                for dt in range(CT):
                    pmt = psO.tile([128, BS], F32, tag="pmt", bufs=2)
                    nc.tensor.matmul(
                        pmt, lhsT=mmy[:, ts(dt, 128)],
                        rhs=identity[0:BS, 0:BS], is_transpose=True,
                        start=True, stop=True, skip_group_check=True)
                    nc.any.tensor_copy(mTmy[dt], pmt)
                for b2 in range(BS // 2):
                    ot = pO.tile([128, 2, CT, HW], F32, tag="ot", bufs=3)
                    for bb in range(2):
                        b = 2 * b2 + bb
                        for dt in range(CT):
                            col = mTmy[dt][:, b:b + 1]
                            if (dt + bb) % 2 == 0:
                                nc.vector.tensor_scalar_mul(
                                    ot[:, bb, dt], ones_hw, col)
                            else:
                                nc.scalar.mul(ot[:, bb, dt], ones_hw, col)
                    nc.sync.dma_start(
                        out=out[2 * b2:2 * b2 + 2].rearrange(
                            "b (ct p) hw -> p b ct hw", p=128),
                        in_=ot)

    nc.compile()
    return nc


_CACHE = {}
TRACE = False
TRACE_DIR = None
LAST_RESULT = None


def _get(shape_key):
    if shape_key not in _CACHE:
        _CACHE[shape_key] = build(*shape_key)
    return _CACHE[shape_key]


def kernel(x, keys, values, topk, **_ignored):
    K = int(np.asarray(topk))
    B, C, H, W = x.shape
    M, D = keys.shape
    HW = H * W
    nc = _get((B, C, HW, M, K, N_CORES))
    BS, MS = B // N_CORES, M // N_CORES
    x3 = np.ascontiguousarray(x.reshape(B, C, HW)).astype(np.float32,
                                                          copy=False)
    keysT = np.ascontiguousarray(keys.T).astype(np.float32, copy=False)
    values = np.ascontiguousarray(values).astype(np.float32, copy=False)
    in_maps = [{
        "xs": x3[c * BS:(c + 1) * BS],
        "keysT": np.ascontiguousarray(keysT[:, c * MS:(c + 1) * MS]),
        "vals": values[c * MS:(c + 1) * MS],
    } for c in range(N_CORES)]
    global LAST_RESULT
    res = run_bass_kernel_spmd(nc, in_maps, core_ids=list(range(N_CORES)),
                               trace=TRACE, tmpdir=TRACE_DIR)
    LAST_RESULT = res
    outs = [res.results[c]["out"] for c in range(N_CORES)]
    return np.concatenate(outs, axis=0).reshape(B, C, H, W)


# revision 41
# speedup vs baseline: 1.1066x; 1.1066x over previous
"""Trainium2 Bass kernel: KV-memory retrieval (pool -> cosine kNN -> softmax gather).

Strategy (8 cores): shard the 65536-slot memory across cores (8192 keys/values
each) and the 256-image batch across cores (32 each) for pooling + output.
Keys are pre-transposed host-side to [C, M] so each core DMAs c-partitioned
kT tiles directly (no PE transposes for keys).

Per core, single SPMD launch:
  1. pool its x shard -> qT columns; two chunked AllGathers (first 16
     batches AG'd while the second half of x still streams in).
     Batch-tile permutation: tile A col r*16+j == global batch r*32+j,
     tile B col r*16+j == global batch r*32+16+j (undone at the mrow
     scatter before ReduceScatter).
  2. stream kT blocks [128c, 512m]: squares (ACT/DVE), norm via f32r
     ones-matmul over partitions, sqrt/recip, PE row-broadcast, DVE
     prescale -> normalized kTn (f32r); f32r matmul1 qT.T @ kTn ->
     sim [256, 8192]; per-block top-8 candidates (max8)
  3. local top-32 -> AllGather candidates -> global top-32 (sorted),
     threshold t, softmax stats gmax / Z (exp with per-partition
     scale/bias)
  4. dense w = exp(sim*rinv + bias) * (sim >= t)  (1/Z folded into bias),
     stored f32r
  5. matmul2 (f32r): values.T @ w -> partial matched.T; values are
     bitcast f32->f32r (no copy)
  6. transpose -> [256, 512], ReduceScatter(add) -> own batch shard
  7. broadcast over 784 spatial positions, DMA out [32, 512, 784]

f32r (single-pass fp32 on the PE) is safe here: measured sim noise ~1e-6
in cos units vs ~4e-4 gaps between rank 32/33. Selection is done on raw
r = q_sum . k_norm (scale-invariant per batch row); 1/||q|| enters only
through the exp scale. Mean /784 cancels everywhere.
"""

import math

import numpy as np

import concourse.bacc as bacc
import concourse.mybir as mybir
import concourse.tile as tile
from concourse.bass import ts
from concourse.bass_utils import run_bass_kernel_spmd
from concourse.masks import make_identity

F32 = mybir.dt.float32
F32R = mybir.dt.float32r
AF = mybir.ActivationFunctionType
ALU = mybir.AluOpType

N_CORES = 8
NEG = -3.0e38


def build(B=256, C=512, HW=784, M=65536, K=32, n_cores=N_CORES, mb=512):
    """Build + bacc-compile the SPMD program. Returns nc."""
    BS = B // n_cores          # batches per core
    HB = BS // 2               # half-batch chunk for split AllGather
    MS = M // n_cores          # memory slots per core
    CT = C // 128              # channel tiles (also contraction tiles)
    BT = B // 128 if B >= 128 else 1
    BTW = 128 if B >= 128 else B   # batch-tile width
    assert B % BTW == 0 and C % 128 == 0 and M % (n_cores * mb) == 0
    NMB = MS // mb             # key blocks per core
    R = math.ceil(K / 8)       # max8 rounds for exact top-K
    KPB = 8                    # top-8 per 512-block (validated sufficient)
    MT = MS // 128             # value tiles
    RG = [list(range(n_cores))]
    CC_AS = "Shared" if n_cores > 4 else "Local"

    nc = bacc.Bacc("TRN2", target_bir_lowering=False, debug=False,
                   num_devices=n_cores)

    xs = nc.dram_tensor("xs", [BS, C, HW], F32, kind="ExternalInput").ap()
    keysT = nc.dram_tensor("keysT", [C, MS], F32, kind="ExternalInput").ap()
    vals = nc.dram_tensor("vals", [MS, C], F32, kind="ExternalInput").ap()
    out = nc.dram_tensor("out", [BS, C, HW], F32, kind="ExternalOutput").ap()

    with tile.TileContext(nc) as tc:
        with (
            tc.tile_pool(name="consts", bufs=1) as consts,
            tc.tile_pool(name="persist", bufs=1) as persist,
            tc.tile_pool(name="dram", bufs=1, space="DRAM") as dram,
        ):
            identity = consts.tile([128, 128], F32)
            make_identity(nc, identity)
            identity_r = consts.tile([128, 128], F32R)
            nc.vector.tensor_copy(identity_r, identity)
            ones_col = consts.tile([128, 1], F32)
            nc.vector.memset(ones_col, 1.0)
            ones_col_r = consts.tile([128, 1], F32R)
            nc.vector.tensor_copy(ones_col_r, ones_col)
            ones_row = consts.tile([1, 128], F32)
            nc.vector.memset(ones_row, 1.0)
            ones_row_r = consts.tile([1, 128], F32R)
            nc.vector.tensor_copy(ones_row_r, ones_row)
            ones_hw = consts.tile([128, HW], F32)
            nc.vector.memset(ones_hw, 1.0)
            identity_h = consts.tile([128, 128], mybir.dt.bfloat16)
            nc.vector.tensor_copy(identity_h, identity)

            sim = [persist.tile([BTW, MS], F32, name=f"sim{i}")
                   for i in range(BT)]
            qTt = persist.tile([128, CT, B], F32, name="qTt")
            qT = [qTt[:, i] for i in range(CT)]
            qTr = persist.tile([128, CT, B], F32R, name="qTr")
            qTl = [persist.tile([128, BS], F32, name=f"qTl{i}")
                   for i in range(CT)]
            cand = [persist.tile([BTW, NMB * KPB], F32, name=f"cand{i}")
                    for i in range(BT)]
            rinv = [persist.tile([BTW, 1], F32, name=f"rinv{i}")
                    for i in range(BT)]
            bias2 = [persist.tile([BTW, 1], F32, name=f"bias2{i}")
                     for i in range(BT)]
            g32 = [persist.tile([BTW, R * 8], F32, name=f"g32{i}")
                   for i in range(BT)]
            mrow = [persist.tile([BTW, C], F32, name=f"mrow{i}")
                    for i in range(BT)]
            mTmy = [persist.tile([128, BS], F32, name=f"mTmy{i}")
                    for i in range(CT)]

            # AllGather staging (queries). A single AG after pooling: an
            # early chunked AG measurably steals x-read DMA bandwidth and
            # its second chunk lands late (rank skew), stalling the PE
            # queue; key_prep needs no queries, so it fills the AG window.
            qag_in = dram.tile([C, BS], F32, name="qag_in")
            qag_out = dram.tile([n_cores, C, BS], F32, addr_space=CC_AS,
                                name="qag_out")

            def ag_queries():
                for ct in range(CT):
                    nc.scalar.dma_start(out=qag_in[ts(ct, 128), :],
                                        in_=qTl[ct])
                nc.gpsimd.collective_compute(
                    "AllGather", ALU.bypass, replica_groups=RG,
                    ins=[qag_in.opt()], outs=[qag_out.opt()])

            def unpack_queries():
                # on the SP queue: SP would self-block on ktb slot pacing
                # here anyway, while ACT must keep running key_prep work
                # through the AllGather window
                for r in range(n_cores):
                    nc.sync.dma_start(
                        out=qTt[:, :, r * BS:(r + 1) * BS],
                        in_=qag_out[r].rearrange("(ct p) b -> p ct b",
                                                 p=128))
                nc.vector.tensor_copy(qTr, qTt)

            # ---------------- Phase P: pool x -> qT local ----------------
            with (
                tc.tile_pool(name="poolP", bufs=3) as pP,
            ):
                hw_a = 0
                for a in range(int(math.isqrt(HW)), 1, -1):
                    if HW % a == 0:
                        hw_a = a
                        break
                CTH = CT // 2
                for b in range(BS):
                    if b % 2 == 0:
                        xt2 = pP.tile([128, 2, CT, HW], F32, tag="xt2",
                                      bufs=3)
                        nc.sync.dma_start(
                            out=xt2,
                            in_=xs[b:b + 2].rearrange(
                                "b (ct p) hw -> p b ct hw", p=128))
                    xt = xt2[:, b % 2]
                    if hw_a > 1:
                        xp = pP.tile([128, CTH, HW // hw_a], F32, tag="xp")
                        nc.vector.tensor_reduce(
                            out=xp,
                            in_=xt[:, 0:CTH].rearrange(
                                "p ct (a b) -> p ct a b", a=HW // hw_a),
                            axis=mybir.AxisListType.X, op=ALU.add)
                        xq = pP.tile([128, CTH], F32, tag="xq")
                        nc.vector.tensor_reduce(
                            out=xq, in_=xp,
                            axis=mybir.AxisListType.X, op=ALU.add)
                    else:
                        xq = pP.tile([128, CTH], F32, tag="xq")
                        nc.vector.tensor_reduce(
                            out=xq, in_=xt[:, 0:CTH],
                            axis=mybir.AxisListType.X, op=ALU.add)
                    for ct in range(CTH):
                        nc.vector.tensor_copy(qTl[ct][:, b:b + 1],
                                              xq[:, ct:ct + 1])
                    for ct in range(CTH, CT):
                        xsc = pP.tile([128, HW], F32, tag="xsc")
                        nc.scalar.activation(
                            xsc, xt[:, ct], AF.Copy,
                            accum_out=qTl[ct][:, b:b + 1])
                ag_queries()

            # ---------------- Phase K: keys -> sim + block candidates -----
            # value prefetch pool spans K+W so DMA never starves after keys
            VB = 4                      # value tiles per DMA
            CW = 4                      # wexp chunk width (value tiles)
            pV_cm = tc.tile_pool(name="poolV", bufs=2)
            pV = pV_cm.__enter__()
            vgroups = {}

            def issue_vals(g):
                vtb = pV.tile([128, VB, C], F32R, tag="vtb", bufs=5)
                nc.sync.dma_start(
                    out=vtb,
                    in_=vals[g * VB * 128:(g + 1) * VB * 128].rearrange(
                        "(v p) c -> p v c", p=128).bitcast(F32R))
                vgroups[g] = vtb

            with (
                tc.tile_pool(name="poolK", bufs=2) as pK,
                tc.tile_pool(name="psumK", bufs=1, space="PSUM") as psK,
            ):
                ktbs = {}
                psbSs = {}

                def key_prep(mbi):
                    # load kT block [128, CT, mb] (c-partitioned, direct),
                    # typed f32r via bitcast (PE truncates internally; the
                    # per-element noise averages out in the dot product)
                    ktb = pK.tile([128, CT, mb], F32R, tag="ktb", bufs=6)
                    nc.sync.dma_start(
                        out=ktb,
                        in_=keysT[:, mbi * mb:(mbi + 1) * mb].rearrange(
                            "(ct p) m -> p ct m", p=128).bitcast(F32R))
                    ktbs[mbi] = ktb
                    # squares -> ksq (f32r-rounded, so the f32r norm matmul
                    # sums exactly-representable terms: ~8e-6 norm error)
                    ksq = pK.tile([128, CT, mb], F32R, tag="ksq", bufs=2)
                    for ct in range(CT):
                        if ct == 0:
                            nc.vector.tensor_mul(ksq[:, ct], ktb[:, ct],
                                                 ktb[:, ct])
                        else:
                            nc.scalar.square(ksq[:, ct], ktb[:, ct])
                    # norms^2: contract partitions via ones f32r matmul
                    psn = psK.tile([1, mb], F32, tag="psn", bufs=2)
                    for ct in range(CT):
                        nc.tensor.matmul(psn, lhsT=ones_col_r,
                                         rhs=ksq[:, ct],
                                         start=(ct == 0), stop=(ct == CT - 1),
                                         skip_group_check=True)
                    nrow = pK.tile([1, mb], F32, tag="nrow", bufs=2)
                    nc.scalar.sqrt(nrow, psn)
                    rrow = pK.tile([1, mb], F32, tag="rrow", bufs=2)
                    nc.vector.reciprocal(rrow, nrow)
                    # broadcast rinv row to 128 partitions on the PE in FULL
                    # fp32 (1.0 * x is exact; f32r here would round the
                    # per-key scale and flip rank-32/33 boundaries)
                    psb = psK.tile([128, mb], F32, tag="psb", bufs=2)
                    nc.tensor.matmul(psb, lhsT=ones_row, rhs=rrow,
                                     start=True, stop=True,
                                     skip_group_check=True)
                    psbS = pK.tile([128, mb], F32, tag="psbS", bufs=6)
                    nc.scalar.copy(psbS, psb)
                    psbSs[mbi] = psbS

                def sim_block(mbi, bts):
                    ktb = ktbs[mbi]
                    psbS = psbSs[mbi]
                    for bt in bts:
                        psim = psK.tile([BTW, mb], F32, tag="psim", bufs=4)
                        for dt in range(CT):
                            nc.tensor.matmul(
                                psim, lhsT=qTr[:, dt, ts(bt, BTW)],
                                rhs=ktb[:, dt],
                                start=(dt == 0), stop=(dt == CT - 1),
                                skip_group_check=True)
                        # evict + normalize by 1/||k|| in one DVE op
                        sblk = sim[bt][:, ts(mbi, mb)]
                        nc.vector.tensor_mul(sblk, psim, psbS)
                        nc.vector.max(
                            cand[bt][:, mbi * KPB:mbi * KPB + KPB], sblk)

                # key DMA + norm chains for the first PF blocks fill the
                # AllGather window (no dependency on queries)
                PF = 4
                for mbi in range(PF):
                    key_prep(mbi)
                unpack_queries()
                for mbi in range(NMB):
                    if mbi + PF < NMB:
                        key_prep(mbi + PF)
                    if mbi >= NMB - 4:
                        issue_vals(mbi - (NMB - 4))
                    sim_block(mbi, list(range(BT)))
                ktbs.clear()
                psbSs.clear()

            # ---------------- Phase Q: query norms ----------------
            with (
                tc.tile_pool(name="poolQ", bufs=2) as pQ,
                tc.tile_pool(name="psumQ", bufs=1, space="PSUM") as psQ,
            ):
                psum_ssq = psQ.tile([1, B], F32, tag="ssq")
                for ct in range(CT):
                    qsq = pQ.tile([128, B], F32R, tag="qsq")
                    nc.scalar.square(qsq, qT[ct])
                    nc.tensor.matmul(psum_ssq, lhsT=ones_col_r, rhs=qsq,
                                     start=(ct == 0), stop=(ct == CT - 1))
                qn_row = pQ.tile([1, B], F32, tag="qn_row", bufs=1)
                nc.scalar.sqrt(qn_row, psum_ssq)
                ri_row = pQ.tile([1, B], F32, tag="ri_row", bufs=1)
                nc.vector.reciprocal(ri_row, qn_row)
                for bt in range(BT):
                    psum_rt = psQ.tile([BTW, 1], F32, tag="rt")
                    nc.tensor.matmul(
                        psum_rt, lhsT=ri_row[0:1, ts(bt, BTW)],
                        rhs=ones_col[0:1, 0:1], start=True, stop=True)
                    nc.vector.tensor_copy(rinv[bt], psum_rt)

            # ---------------- Phase G + W: top-K stats, dense matmul2 -----
            cd_in = dram.tile([B, K], F32)
            cd_out = dram.tile([n_cores, B, K], F32, addr_space=CC_AS)
            with (
                tc.tile_pool(name="poolW", bufs=2) as pW,
                tc.tile_pool(name="psumW", bufs=1, space="PSUM") as psW,
                tc.tile_pool(name="poolG", bufs=1) as pG,
            ):
                # G1: local top-K per bt -> AllGather (bt chains interleaved
                # so the DVE pipeline stays full)
                locs = [pG.tile([BTW, R * 8], F32, tag=f"loc{bt}",
                                name=f"loc{bt}") for bt in range(BT)]
                scr2s = [pG.tile([BTW, NMB * KPB], F32, tag=f"scr2{bt}",
                                 name=f"scr2{bt}") for bt in range(BT)]
                curs = [cand[bt] for bt in range(BT)]
                for r in range(R):
                    for bt in range(BT):
                        nc.vector.max(locs[bt][:, r * 8:(r + 1) * 8],
                                      curs[bt])
                    if r < R - 1:
                        for bt in range(BT):
                            nc.vector.match_replace(
                                scr2s[bt],
                                in_to_replace=locs[bt][:, r * 8:(r + 1) * 8],
                                in_values=curs[bt], imm_value=NEG)
                            curs[bt] = scr2s[bt]
                for bt in range(BT):
                    nc.sync.dma_start(out=cd_in[ts(bt, BTW), :],
                                      in_=locs[bt][:, 0:K])
                nc.gpsimd.collective_compute(
                    "AllGather", ALU.bypass, replica_groups=RG,
                    ins=[cd_in.opt()], outs=[cd_out.opt()])

                issue_vals(4)
                issue_vals(5)

                # G2: global top-K + softmax stats. bt chains interleaved;
                # Exp/Ln activations grouped to avoid ACT table thrash.
                gcs, scr3s = [], []
                for bt in range(BT):
                    gc = pG.tile([BTW, n_cores * K], F32, tag=f"gc{bt}",
                                 name=f"gc{bt}")
                    nc.scalar.dma_start(
                        out=gc,
                        in_=cd_out[:, ts(bt, BTW), :].rearrange(
                            "r b k -> b r k"))
                    gcs.append(gc)
                    scr3s.append(pG.tile([BTW, n_cores * K], F32,
                                         tag=f"scr3{bt}", name=f"scr3{bt}"))
                curs = gcs
                for r in range(R):
                    for bt in range(BT):
                        nc.vector.max(g32[bt][:, r * 8:(r + 1) * 8],
                                      curs[bt])
                    if r < R - 1:
                        for bt in range(BT):
                            nc.vector.match_replace(
                                scr3s[bt],
                                in_to_replace=g32[bt][:, r * 8:(r + 1) * 8],
                                in_values=curs[bt], imm_value=NEG)
                        curs = scr3s
                # stats: nb = -gmax*rinv ; Z = sum exp((g - gmax)*rinv)
                nbs, zzs = [], []
                for bt in range(BT):
                    nb = pG.tile([BTW, 1], F32, tag=f"nb{bt}",
                                 name=f"nb{bt}")
                    nc.vector.tensor_mul(nb, g32[bt][:, 0:1], rinv[bt])
                    nc.vector.tensor_scalar_mul(nb, nb, -1.0)
                    nbs.append(nb)
                for bt in range(BT):
                    ex = pG.tile([BTW, K], F32, tag="ex")
                    zz = pG.tile([BTW, 1], F32, tag=f"zz{bt}",
                                 name=f"zz{bt}")
                    nc.scalar.activation(ex, g32[bt][:, 0:K], AF.Exp,
                                         bias=nbs[bt], scale=rinv[bt],
                                         accum_out=zz)
                    zzs.append(zz)
                lnzs = []
                for bt in range(BT):
                    lnz = pG.tile([BTW, 1], F32, tag=f"lnz{bt}",
                                  name=f"lnz{bt}")
                    nc.scalar.activation(lnz, zzs[bt], AF.Ln)
                    lnzs.append(lnz)
                for bt in range(BT):
                    nc.vector.tensor_sub(bias2[bt], nbs[bt], lnzs[bt])

                # W: dense weights + matmul2
                pmB = [psW.tile([BTW, C], F32, tag=f"pmB{bt}",
                                name=f"pmB{bt}") for bt in range(BT)]
                for mt in range(MT):
                    g = mt // VB
                    if mt % VB == 0 and g + 3 not in vgroups and \
                            (g + 3) * VB * 128 < MS:
                        issue_vals(g + 3)
                    if mt % CW == 0:
                        # weights quantized to bf16 after the f32 threshold
                        # compare: 2^-9 relative on softmax weights is far
                        # below the tolerance, and bf16 transposes run the
                        # PE at 1 cyc/row with fast (FWL) weight loads.
                        weF = [pW.tile([BTW, CW * 128], F32R, tag=f"weF{bt}",
                                       bufs=2, name=f"weF{bt}")
                               for bt in range(BT)]
                        we = [pW.tile([BTW, CW * 128], mybir.dt.bfloat16,
                                      tag=f"we{bt}", bufs=2,
                                      name=f"we{bt}")
                              for bt in range(BT)]
                        for bt in range(BT):
                            schunk = sim[bt][:, mt * 128:(mt + CW) * 128]
                            nc.scalar.activation(weF[bt], schunk, AF.Exp,
                                                 bias=bias2[bt],
                                                 scale=rinv[bt])
                            nc.vector.scalar_tensor_tensor(
                                out=we[bt], in0=schunk,
                                scalar=g32[bt][:, K - 1:K], in1=weF[bt],
                                op0=ALU.is_ge, op1=ALU.mult)
                    vt = vgroups[g][:, mt % VB]
                    off = (mt % CW) * 128
                    pwt = psW.tile([128, B], mybir.dt.bfloat16, tag="pwt",
                                   bufs=4)
                    for bt in range(BT):
                        nc.tensor.matmul(
                            pwt[:, ts(bt, BTW)],
                            lhsT=we[bt][:, off:off + 128],
                            rhs=identity_h[0:BTW, 0:BTW], is_transpose=True,
                            start=True, stop=True, skip_group_check=True)
                    wT = pW.tile([128, B], F32R, tag="wT", bufs=3)
                    if mt % 2 == 0:
                        nc.vector.tensor_copy(wT, pwt)
                    else:
                        nc.scalar.copy(wT, pwt)
                    for bt in range(BT):
                        nc.tensor.matmul(
                            pmB[bt], lhsT=wT[:, ts(bt, BTW)], rhs=vt,
                            start=(mt == 0), stop=(mt == MT - 1),
                            skip_group_check=True)
                for bt in range(BT):
                    nc.any.tensor_copy(mrow[bt], pmB[bt])
            pV_cm.__exit__(None, None, None)

            # ---------------- Phase O: reduce-scatter + broadcast out -----
            mb_dram = dram.tile([B, C], F32)
            rs_out = dram.tile([BS, C], F32)
            with (
                tc.tile_pool(name="poolO", bufs=2) as pO,
                tc.tile_pool(name="psumO", bufs=1, space="PSUM") as psO,
            ):
                for bt in range(BT):
                    nc.sync.dma_start(out=mb_dram[ts(bt, BTW), :],
                                      in_=mrow[bt])
                nc.gpsimd.collective_compute(
                    "ReduceScatter", ALU.add, replica_groups=RG,
                    ins=[mb_dram.opt()], outs=[rs_out.opt()])
                mmy = pO.tile([BS, C], F32, tag="mmy", bufs=1)
                nc.scalar.dma_start(out=mmy, in_=rs_out)
                for dt in range(CT):
                    pmt = psO.tile([128, BS], F32, tag="pmt", bufs=2)
                    nc.tensor.matmul(
                        pmt, lhsT=mmy[:, ts(dt, 128)],
                        rhs=identity[0:BS, 0:BS], is_transpose=True,
                        start=True, stop=True, skip_group_check=True)
                    nc.any.tensor_copy(mTmy[dt], pmt)
                for b2 in range(BS // 2):
                    ot = pO.tile([128, 2, CT, HW], F32, tag="ot", bufs=3)
                    for bb in range(2):
                        b = 2 * b2 + bb
                        for dt in range(CT):
                            col = mTmy[dt][:, b:b + 1]
                            if (dt + bb) % 2 == 0:
                                nc.vector.tensor_scalar_mul(
                                    ot[:, bb, dt], ones_hw, col)
                            else:
                                nc.scalar.mul(ot[:, bb, dt], ones_hw, col)
                    nc.sync.dma_start(
                        out=out[2 * b2:2 * b2 + 2].rearrange(
                            "b (ct p) hw -> p b ct hw", p=128),
                        in_=ot)

    nc.compile()
    return nc


_CACHE = {}
TRACE = False
TRACE_DIR = None
LAST_RESULT = None


def _get(shape_key):
    if shape_key not in _CACHE:
        _CACHE[shape_key] = build(*shape_key)
    return _CACHE[shape_key]


def kernel(x, keys, values, topk, **_ignored):
    K = int(np.asarray(topk))
    B, C, H, W = x.shape
    M, D = keys.shape
    HW = H * W
    nc = _get((B, C, HW, M, K, N_CORES))
    BS, MS = B // N_CORES, M // N_CORES
    x3 = np.ascontiguousarray(x.reshape(B, C, HW)).astype(np.float32,
                                                          copy=False)
    keysT = np.ascontiguousarray(keys.T).astype(np.float32, copy=False)
    values = np.ascontiguousarray(values).astype(np.float32, copy=False)
    in_maps = [{
        "xs": x3[c * BS:(c + 1) * BS],
        "keysT": np.ascontiguousarray(keysT[:, c * MS:(c + 1) * MS]),
        "vals": values[c * MS:(c + 1) * MS],
    } for c in range(N_CORES)]
    global LAST_RESULT
    res = run_bass_kernel_spmd(nc, in_maps, core_ids=list(range(N_CORES)),
                               trace=TRACE, tmpdir=TRACE_DIR)
    LAST_RESULT = res
    outs = [res.results[c]["out"] for c in range(N_CORES)]
    return np.concatenate(outs, axis=0).reshape(B, C, H, W)
